# revision 3
# baseline (speedup 1.0000x reference)
"""FP8 delayed-scaling Linear (8192x4096 @ 4096x4096^T + bias) on 8 NeuronCores.

Strategy: 2D tensor-parallel sharding: token dim (T=8192) split 4 ways x
out_features (O=4096) split 2 ways -> 8 independent cores, no collectives
(the amax max-all-reduce happens in the host-side gather).

Numerics: the reference quantizes with OCP float8_e4m3fn (max 448). TRN2's
fp8e4 has max 240, so we quantize with the HALVED scale (s/2 = 224/amax) and
clip at +-224: every OCP grid point v with |v| <= 448 maps to v/2 which is
exactly representable in TRN fp8e4 (verified: 119 of 4M random normals
mismatch, all |x|<7e-5 with ~4e-6 abs error). The matmul output is then
descaled by 4/(sx*sw) with the bias fused into PSUM eviction.
"""

import numpy as np

import concourse.bass as bass
import concourse.bacc as bacc
import concourse.mybir as mybir
import concourse.tile as tile
from concourse import bass_utils

P = 128
FP8_MAX = 448.0
AMAX_EPS = 1e-8
MOMENTUM = 0.95
CLIP = 224.0  # 448/2 in scaled units

N_CORES = 8
A_SHARD = 4  # split of T (token rows)
B_SHARD = 2  # split of O (out features)


def build_kernel(nc, K, T_loc, O_loc, sx2, sw2, descale):
    """Per-core kernel. Inputs (DRAM): xT [K, T_loc] f32, wT [K, O_loc] f32,
    bias [O_loc] f32. Outputs: out [T_loc, O_loc] f32,
    stats [128, 2] f32 (per-partition max|sx2*x|, max|sw2*w|)."""
    f32 = mybir.dt.float32
    f8 = mybir.dt.float8e4
    Alu = mybir.AluOpType
    Act = mybir.ActivationFunctionType

    assert K % (2 * P) == 0 and T_loc % P == 0 and O_loc % 512 == 0
    KS = K // P          # k-subtiles of 128
    KP = KS // 2         # k-pair slabs (DoubleRow consumes 2 subtiles)
    MT = T_loc // P      # t-strips
    NB = O_loc // 512    # 512-wide n-blocks (one PSUM bank each)

    xT = nc.dram_tensor("xT", (K, T_loc), f32, kind="ExternalInput")
    wT = nc.dram_tensor("wT", (K, O_loc), f32, kind="ExternalInput")
    bias = nc.dram_tensor("bias", (O_loc,), f32, kind="ExternalInput")
    out = nc.dram_tensor("out", (T_loc, O_loc), f32, kind="ExternalOutput")
    stats = nc.dram_tensor("stats", (P, 2), f32, kind="ExternalOutput")

    xT_r = xT.ap().rearrange("(ks p) t -> p ks t", p=P)            # [128,KS,T_loc]
    wT_r = wT.ap().rearrange("(kp s p) o -> kp p s o", s=2, p=P)   # [KP,128,2,O_loc]
    out_r = out.ap().rearrange("(mt p) o -> mt p o", p=P)          # [MT,128,O_loc]

    with tile.TileContext(nc) as tc:
        with (
            tc.tile_pool(name="const", bufs=1) as const,
            tc.tile_pool(name="stage", bufs=3) as stage,
            tc.tile_pool(name="w8pool", bufs=KP) as w8pool,
            tc.tile_pool(name="x8pool", bufs=3) as x8pool,
            tc.tile_pool(name="outp", bufs=2) as outp,
            tc.tile_pool(name="psum", bufs=8, space="PSUM") as psum,
        ):
            # bias broadcast to all partitions
            bias_p0 = const.tile([P, O_loc], f32, name="bias_p0")
            bias_sb = const.tile([P, O_loc], f32, name="bias_sb")
            nc.sync.dma_start(bias_p0[0:1, :], bias.ap())
            nc.gpsimd.partition_broadcast(bias_sb[:], bias_p0[0:1, :])

            xpart = const.tile([P, MT], f32, name="xpart")
            wpart = const.tile([P, KP], f32, name="wpart")

            # ---- W: load k-pair slabs, quantize, keep fp8 resident ----
            w8 = []
            for j in range(KP):
                stg = stage.tile([P, 2, O_loc], f32, name="stg")
                nc.sync.dma_start(stg[:], wT_r[j])
                nc.scalar.activation(stg[:], stg[:], Act.Copy, scale=sw2)
                nc.vector.tensor_reduce(
                    wpart[:, j : j + 1], stg[:], axis=mybir.AxisListType.XY,
                    op=Alu.max, apply_absolute_value=True,
                )
                t8 = w8pool.tile([P, 2, O_loc], f8, name="w8")
                nc.vector.tensor_scalar(
                    t8[:], stg[:], CLIP, -CLIP, op0=Alu.min, op1=Alu.max
                )
                w8.append(t8)

            # ---- X strips: quantize, matmul against all of W ----
            for m in range(MT):
                stg = stage.tile([P, KS, P], f32, name="stg")
                nc.sync.dma_start(stg[:], xT_r[:, :, m * P : (m + 1) * P])
                nc.scalar.activation(stg[:], stg[:], Act.Copy, scale=sx2)
                nc.vector.tensor_reduce(
                    xpart[:, m : m + 1], stg[:], axis=mybir.AxisListType.XY,
                    op=Alu.max, apply_absolute_value=True,
                )
                x8 = x8pool.tile([P, KS, P], f8, name="x8")
                nc.vector.tensor_scalar(
                    x8[:], stg[:], CLIP, -CLIP, op0=Alu.min, op1=Alu.max
                )

                psums = [
                    psum.tile([P, 512], f32, name="ps") for n in range(NB)
                ]
                for j in range(KP):
                    lhsT = x8[:, 2 * j : 2 * j + 2, :]
                    for n in range(NB):
                        nc.tensor.matmul(
                            psums[n][:],
                            lhsT,
                            w8[j][:, :, n * 512 : (n + 1) * 512],
                            start=(j == 0),
                            stop=(j == KP - 1),
                            perf_mode=mybir.MatmulPerfMode.DoubleRow,
                        )

                outm = outp.tile([P, O_loc], f32, name="outm")
                for n in range(NB):
                    nc.vector.scalar_tensor_tensor(
                        outm[:, n * 512 : (n + 1) * 512],
                        psums[n][:],
                        descale,
                        bias_sb[:, n * 512 : (n + 1) * 512],
                        op0=Alu.mult,
                        op1=Alu.add,
                    )
                nc.sync.dma_start(out_r[m], outm[:])

            # ---- final per-partition stats ----
            st = const.tile([P, 2], f32, name="st")
            nc.vector.tensor_reduce(
                st[:, 0:1], xpart[:], axis=mybir.AxisListType.X, op=Alu.max
            )
            nc.vector.tensor_reduce(
                st[:, 1:2], wpart[:], axis=mybir.AxisListType.X, op=Alu.max
            )
            nc.sync.dma_start(stats.ap(), st[:])
    return nc


def _amax_update(buf, new_amax):
    new_amax = np.float32(new_amax)
    if not np.isfinite(new_amax):
        new_amax = np.float32(AMAX_EPS if not new_amax > 0 else FP8_MAX)
    return np.float32(
        np.clip(np.maximum(np.float32(buf) * np.float32(MOMENTUM), new_amax),
                np.float32(AMAX_EPS), None)
    )


def kernel(x, weight, bias, input_amax, weight_amax):
    x = np.asarray(x, dtype=np.float32)
    weight = np.asarray(weight, dtype=np.float32)
    bias = np.asarray(bias, dtype=np.float32)
    input_amax = np.float32(np.asarray(input_amax))
    weight_amax = np.float32(np.asarray(weight_amax))

    T, K = x.shape
    O = weight.shape[0]
    T_loc, O_loc = T // A_SHARD, O // B_SHARD

    # scales exactly as the reference computes them (f32), then halved (exact)
    sx = np.float32(FP8_MAX) / np.float32(np.clip(input_amax, AMAX_EPS, None))
    sw = np.float32(FP8_MAX) / np.float32(np.clip(weight_amax, AMAX_EPS, None))
    sx2 = np.float32(sx * np.float32(0.5))
    sw2 = np.float32(sw * np.float32(0.5))
    descale = np.float32(1.0 / (np.float64(sx2) * np.float64(sw2)))

    xT = np.ascontiguousarray(x.T)        # [K, T]
    wT = np.ascontiguousarray(weight.T)   # [K, O]

    in_maps = []
    for c in range(N_CORES):
        ai, bi = divmod(c, B_SHARD)
        in_maps.append({
            "xT": xT[:, ai * T_loc : (ai + 1) * T_loc],
            "wT": wT[:, bi * O_loc : (bi + 1) * O_loc],
            "bias": bias[bi * O_loc : (bi + 1) * O_loc],
        })

    nc = bacc.Bacc("TRN2", target_bir_lowering=False, debug=False,
                   num_devices=N_CORES)
    build_kernel(nc, K, T_loc, O_loc, float(sx2), float(sw2), float(descale))
    nc.compile()
    res = bass_utils.run_bass_kernel_spmd(nc, in_maps, core_ids=list(range(N_CORES)))

    out = np.empty((T, O), dtype=np.float32)
    xmax_s = np.float32(0.0)
    wmax_s = np.float32(0.0)
    for c in range(N_CORES):
        ai, bi = divmod(c, B_SHARD)
        r = res.results[c]
        out[ai * T_loc : (ai + 1) * T_loc, bi * O_loc : (bi + 1) * O_loc] = r["out"]
        st = r["stats"]
        xmax_s = max(xmax_s, st[:, 0].max())
        wmax_s = max(wmax_s, st[:, 1].max())

    amax_x = np.float32(np.float64(xmax_s) / np.float64(sx2))
    amax_w = np.float32(np.float64(wmax_s) / np.float64(sw2))
    new_input_amax = _amax_update(input_amax, amax_x)
    new_weight_amax = _amax_update(weight_amax, amax_w)
    return out, new_input_amax, new_weight_amax
